# revision 3
# baseline (speedup 1.0000x reference)
"""STConvBlock Trainium2 kernel v2: bf16 compute, resident masks, PE-summed
scores, exact union-mask correction via (u-1) matmul.

Sharding: 40 (slice, head) units; core c owns units [4c..4c+3, 32+c] so that
local units 0,1 share slice A (heads 0,1), 2,3 share slice B, 4 is an extra
slice. AllGather split in two (32-unit, 8-unit) to overlap unit 4's compute.
Tail (tconv2 + norm) replicated on every core; out stored [B,T2,C,N], host
transposes.

Score math per (slice xs, head, cheb k), tile [128 j, 1024 i] (S transposed):
  v_r = al_r[i] + ar_r[j]    (DVE tensor_scalar: al broadcast + per-part ar)
  t_r = v_r * m_r            (DVE tensor_tensor, masks resident bf16)
  ss  = t0+t1+t2             (PE: 3 identity-matmul injects into f32 PSUM)
  X   = exp(ss)              (ACT, PSUM -> SBUF bf16; off-union X == 1.0)
  num/den += wxo^T @ X + wxo^T @ (u-1)   (PE; exact cancel off-union)
"""

import os
import numpy as np
import ml_dtypes

B, T, N, C = 2, 12, 1024, 64
KT = 3
T1 = T - KT + 1   # 10
T2 = T1 - KT + 1  # 8
H, K1, R = 2, 3, 2
NSLICE = B * T1       # 20
NUNITS = NSLICE * H   # 40
N_CORES = 8
NT = N // 128         # 8
FCH = 512
NF = N // FCH         # 2
NC_ELEMS = float(N * C)
UPC = NUNITS // N_CORES  # 5
NSL = 3                  # distinct slices per core
# local unit j -> slice index (uniform across cores by construction)
J2S = [0, 0, 1, 1, 2]
J2H = [0, 1, 0, 1, None]  # j=4 head is core-dependent (baked into weights)

_cache = {}


def _build(n_cores, triv_gb=True, use_umul=False, debug=False):
    import concourse.bass as bass
    import concourse.tile as tile
    import concourse.mybir as mybir
    from concourse import bacc
    from concourse.masks import make_identity

    F32 = mybir.dt.float32
    BF16 = mybir.dt.bfloat16
    FP8 = mybir.dt.float8e4
    AF = mybir.ActivationFunctionType
    ALU = mybir.AluOpType
    AX = mybir.AxisListType

    nc = bacc.Bacc(None, target_bir_lowering=False)
    xw = nc.dram_tensor("xw", [NSL, C, KT, N], BF16, kind="ExternalInput")
    w1T = nc.dram_tensor("w1T", [KT, C, 2 * C], BF16, kind="ExternalInput")
    w2T = nc.dram_tensor("w2T", [KT, C, 2 * C], BF16, kind="ExternalInput")
    res1 = nc.dram_tensor("res1", [C, 2 * C], BF16, kind="ExternalInput")
    res05 = nc.dram_tensor("res05", [C, 2 * C], BF16, kind="ExternalInput")
    wlr = nc.dram_tensor("wlr", [UPC, K1, C, K1], BF16, kind="ExternalInput")
    # wrt = [Wt@Wr.T | Wt] concat: one matmul yields ar [.,0:3] and wx [.,3:67]
    wrt = nc.dram_tensor("wrt", [UPC, K1, C, K1 + C], BF16, kind="ExternalInput")
    mrelT = nc.dram_tensor("mrelT", [R, N, N], BF16, kind="ExternalInput")
    msupT = nc.dram_tensor("msupT", [K1, N, N], BF16, kind="ExternalInput")
    unegT = nc.dram_tensor("unegT", [K1, N, N], FP8, kind="ExternalInput")
    uT = nc.dram_tensor("uT", [K1, N, N], FP8, kind="ExternalInput")
    gbT = nc.dram_tensor("gbT", [2, C, N], BF16, kind="ExternalInput")
    out = nc.dram_tensor("out", [B, T2, C, N], F32, kind="ExternalOutput")
    ag_in = nc.dram_tensor("ag_in", [UPC, C, N], BF16)
    if debug:
        dxs = nc.dram_tensor("dxs", [NSL, C, N], F32, kind="ExternalOutput")
        dal = nc.dram_tensor("dal", [K1, N], F32, kind="ExternalOutput")
        dar = nc.dram_tensor("dar", [128, K1], F32, kind="ExternalOutput")
        dxe = nc.dram_tensor("dxe", [128, N], F32, kind="ExternalOutput")
        dt0 = nc.dram_tensor("dt0", [128, K1, N], F32, kind="ExternalOutput")
        dop = nc.dram_tensor("dop", [C + 1, N], F32, kind="ExternalOutput")
        dacc = nc.dram_tensor("dacc", [C, N], F32, kind="ExternalOutput")
        drc = nc.dram_tensor("drc", [1, N], F32, kind="ExternalOutput")
        drb = nc.dram_tensor("drb", [C, N], F32, kind="ExternalOutput")
        dnm = nc.dram_tensor("dnm", [C, N], F32, kind="ExternalOutput")
    agos = [nc.dram_tensor(f"ago{j}", [N_CORES, C, N], BF16, addr_space="Shared")
            for j in range(UPC)]

    def agslot(u):  # global unit index -> (tensor, slot)
        return (agos[u % 4], u // 4) if u < 32 else (agos[4], u - 32)

    with tile.TileContext(nc) as tc:
        with (
            tc.tile_pool(name="consts", bufs=1) as consts,
            tc.tile_pool(name="work", bufs=2) as work,
            tc.tile_pool(name="sc", bufs=2) as sc,
            tc.tile_pool(name="ps_s", bufs=2, space="PSUM") as ps_s,
            tc.tile_pool(name="ps_ss", bufs=2, space="PSUM") as ps_ss,
            tc.tile_pool(name="ps_op", bufs=1, space="PSUM") as ps_op,
        ):
            # ---------------- residents ----------------
            w1_sb = consts.tile([C, KT, 2 * C], BF16)
            w2_sb = consts.tile([C, KT, 2 * C], BF16)
            r1_sb = consts.tile([C, 2 * C], BF16)
            r05_sb = consts.tile([C, 2 * C], BF16)
            nc.sync.dma_start(out=w1_sb[:], in_=w1T[:].rearrange("t c o -> c t o"))
            nc.sync.dma_start(out=w2_sb[:], in_=w2T[:].rearrange("t c o -> c t o"))
            nc.sync.dma_start(out=r1_sb[:], in_=res1[:])
            nc.sync.dma_start(out=r05_sb[:], in_=res05[:])
            wlr_sb = consts.tile([C, UPC, K1, K1], BF16)
            wrt_sb = consts.tile([C, UPC, K1, K1 + C], BF16)
            nc.sync.dma_start(out=wlr_sb[:], in_=wlr[:].rearrange("j k c x -> c j k x"))
            nc.sync.dma_start(out=wrt_sb[:], in_=wrt[:].rearrange("j k c x -> c j k x"))
            gb_sb = consts.tile([C, 2, N], BF16)
            nc.sync.dma_start(out=gb_sb[:, 0, :], in_=gbT[0])
            nc.sync.dma_start(out=gb_sb[:, 1, :], in_=gbT[1])
            id128 = consts.tile([128, 128], BF16)
            make_identity(nc, id128)
            ones1x64 = consts.tile([1, C], BF16)
            nc.gpsimd.memset(ones1x64, 1.0)
            ones64x1 = consts.tile([C, 1], F32)
            nc.gpsimd.memset(ones64x1, 1.0)
            ones1x64f = consts.tile([1, C], F32)
            nc.gpsimd.memset(ones1x64f, 1.0)
            eps_sb = consts.tile([1, 1], F32)
            nc.gpsimd.memset(eps_sb, 1e-6)

            # persistent per-jt wx|ones tiles: ones column memset once
            wxo_t = [consts.tile([128, C + 1], BF16, name=f"wxo{jt}")
                     for jt in range(NT)]
            for jt in range(NT):
                nc.gpsimd.memset(wxo_t[jt][:, C : C + 1], 1.0)

            # attention-phase residents: masks + GLU outputs (freed before tail)
            mk = tc.alloc_tile_pool(name="mk", bufs=1)
            xs_sb = mk.tile([C, NSL, N], BF16)  # GLU outputs
            xw_ts = []
            for s in range(NSL):
                xw_t = mk.tile([C, KT, N], BF16, tag="xwp", bufs=2,
                               name=f"xw_t{s}")
                nc.sync.dma_start(out=xw_t[:], in_=xw[s])
                xw_ts.append(xw_t)
            mrel_sb = mk.tile([128, R, NT, N], BF16)
            nc.sync.dma_start(
                out=mrel_sb[:], in_=mrelT[:].rearrange("r (t p) n -> p r t n", p=128)
            )
            msup_k = [mk.tile([128, NT, N], BF16, name=f"msup{k}")
                      for k in range(K1)]
            for k in range(K1):
                nc.sync.dma_start(
                    out=msup_k[k][:],
                    in_=msupT[k].rearrange("(t p) n -> p t n", p=128),
                )

            def glu_conv(w_sb, res_sb, rhs, out_tile, pe_evac_dve=False):
                """out[c,n] = (conv_p + res) * sigmoid(conv_q); res folded via
                identity tap in the matmul (res_sb = [I*scale | 0])."""
                for f in range(NF):
                    cps = ps_ss.tile([2 * C, FCH], F32, tag="ss")
                    for tau in range(KT):
                        nc.tensor.matmul(
                            out=cps, lhsT=w_sb[:, tau, :],
                            rhs=rhs(tau)[:, f * FCH : (f + 1) * FCH],
                            start=(tau == 0), stop=False,
                        )
                    nc.tensor.matmul(
                        out=cps, lhsT=res_sb,
                        rhs=rhs(KT - 1)[:, f * FCH : (f + 1) * FCH],
                        start=False, stop=True,
                    )
                    sg = work.tile([C, FCH], BF16, tag="sg")
                    nc.scalar.activation(out=sg, in_=cps[C:, :], func=AF.Sigmoid)
                    pp = work.tile([C, FCH], BF16, tag="pp")
                    if pe_evac_dve:
                        nc.vector.tensor_copy(out=pp, in_=cps[:C, :])
                    else:
                        nc.scalar.copy(out=pp, in_=cps[:C, :])
                    nc.vector.tensor_mul(
                        out=out_tile[:, f * FCH : (f + 1) * FCH], in0=pp, in1=sg
                    )

            # ---------------- phase G: GLU1 for 3 slices (sigmoid) -------
            for s in range(NSL):
                glu_conv(w1_sb, r1_sb,
                         lambda tau, s=s: xw_ts[s][:, tau, :], xs_sb[:, s, :])
                if debug:
                    dxt = work.tile([C, N], F32, tag="dbgt", bufs=1)
                    nc.vector.tensor_copy(out=dxt, in_=xs_sb[:, s, :])
                    nc.sync.dma_start(out=dxs[s], in_=dxt)

            # ---------------- phase A: attention units (exp) -------------
            for j in range(UPC):
                sdx = J2S[j]
                xsT = xs_sb[:, sdx, :]
                accT = work.tile([C, N], BF16, tag="accT", bufs=1)
                for k in range(K1):
                    # streamed union mask for this k (fp8, [128, NT, N])
                    un_t = work.tile([128, NT, N], FP8, tag="unt",
                                     bufs=1 if debug else 2)
                    nc.sync.dma_start(
                        out=un_t[:],
                        in_=(uT if use_umul else unegT)[k].rearrange(
                            "(t p) n -> p t n", p=128),
                    )
                    # al rows [3, N] and their partition-broadcasts
                    al_sb = work.tile([K1, N], BF16, tag="al_sb", bufs=1)
                    for f in range(NF):
                        alp = ps_s.tile([K1, FCH], F32, tag="alp", bufs=1)
                        nc.tensor.matmul(
                            out=alp, lhsT=wlr_sb[:, j, k, :],
                            rhs=xsT[:, f * FCH : (f + 1) * FCH],
                            start=True, stop=True,
                        )
                        nc.scalar.copy(
                            out=al_sb[:, f * FCH : (f + 1) * FCH], in_=alp
                        )
                    al1_t = work.tile([1, N], BF16, tag="al1_t", bufs=1)
                    al2_t = work.tile([1, N], BF16, tag="al2_t", bufs=1)
                    nc.sync.dma_start(out=al1_t, in_=al_sb[1:2, :])
                    nc.sync.dma_start(out=al2_t, in_=al_sb[2:3, :])
                    al_srcs = [al_sb[0:1, :], al1_t[:], al2_t[:]]
                    dbg_here = debug and j == 0 and k == 0
                    if dbg_here:
                        dalx = work.tile([K1, N], F32, tag="dbgt", bufs=1)
                        nc.vector.tensor_copy(out=dalx, in_=al_sb)
                        nc.sync.dma_start(out=dal[:], in_=dalx)
                    albc = [work.tile([128, N], BF16, tag=f"albc{r}", bufs=1,
                                      name=f"albc{r}") for r in range(K1)]
                    for r in range(K1):
                        nc.gpsimd.partition_broadcast(albc[r][:], al_srcs[r])
                    # [ar | wx] [128, 3+C] per jt in one matmul
                    ar_t = []
                    for jt in range(NT):
                        awp = ps_s.tile([128, K1 + C], F32, tag="awp")
                        nc.tensor.matmul(
                            out=awp, lhsT=xsT[:, jt * 128 : (jt + 1) * 128],
                            rhs=wrt_sb[:, j, k, :], start=True, stop=True,
                        )
                        ar_jt = work.tile([128, K1], F32, tag=f"ar{jt}", bufs=2)
                        nc.scalar.copy(out=ar_jt, in_=awp[:, :K1])
                        ar_t.append(ar_jt)
                        if dbg_here and jt == 0:
                            nc.sync.dma_start(out=dar[:], in_=ar_jt)
                        nc.scalar.copy(out=wxo_t[jt][:, :C], in_=awp[:, K1:])
                    ops = [ps_op.tile([C + 1, FCH], F32, tag=f"op{f}",
                                      name=f"op{f}") for f in range(NF)]
                    for jt in range(NT):
                        ts = []
                        for r in range(K1):
                            vv = sc.tile([128, N], BF16, tag=f"v{r}", bufs=1)
                            nc.vector.tensor_scalar_add(
                                vv, albc[r], ar_t[jt][:, r : r + 1]
                            )
                            tt = sc.tile([128, N], BF16, tag=f"t{r}")
                            msk = (mrel_sb[:, r, jt, :] if r < R
                                   else msup_k[k][:, jt, :])
                            nc.vector.tensor_mul(out=tt, in0=vv, in1=msk)
                            ts.append(tt)
                            if dbg_here and jt == 0:
                                dttx = work.tile([128, N], F32, tag="dbgt", bufs=1)
                                nc.vector.tensor_copy(out=dttx, in_=tt)
                                nc.sync.dma_start(out=dt0[:, r, :], in_=dttx)
                        xe = sc.tile([128, N], BF16, tag="xe")
                        for f in range(NF):
                            ssp = ps_ss.tile([128, FCH], F32, tag="ss")
                            for r in range(K1):
                                nc.tensor.matmul(
                                    out=ssp, lhsT=id128,
                                    rhs=ts[r][:, f * FCH : (f + 1) * FCH],
                                    start=(r == 0), stop=(r == K1 - 1),
                                )
                            nc.scalar.activation(
                                out=xe[:, f * FCH : (f + 1) * FCH], in_=ssp,
                                func=AF.Exp,
                            )
                        if use_umul:
                            nc.vector.tensor_mul(
                                out=xe, in0=xe, in1=un_t[:, jt, :]
                            )
                        if dbg_here and jt == 0:
                            dxex = work.tile([128, N], F32, tag="dbgt", bufs=1)
                            nc.vector.tensor_copy(out=dxex, in_=xe)
                            nc.sync.dma_start(out=dxe[:], in_=dxex)
                        for f in range(NF):
                            nc.tensor.matmul(
                                out=ops[f], lhsT=wxo_t[jt],
                                rhs=xe[:, f * FCH : (f + 1) * FCH],
                                start=(jt == 0), stop=use_umul and (jt == NT - 1),
                            )
                            if not use_umul:
                                nc.tensor.matmul(
                                    out=ops[f], lhsT=wxo_t[jt],
                                    rhs=un_t[:, jt, f * FCH : (f + 1) * FCH],
                                    start=False, stop=(jt == NT - 1),
                                )
                    if dbg_here:
                        for f in range(NF):
                            dopx = work.tile([C + 1, FCH], F32, tag="dbgt", bufs=1)
                            nc.vector.tensor_copy(out=dopx, in_=ops[f])
                            nc.sync.dma_start(
                                out=dop[:, f * FCH : (f + 1) * FCH], in_=dopx)
                    # normalize: accT += num * (1/den) broadcast
                    den_sb = work.tile([1, N], F32, tag="den_sb", bufs=1)
                    for f in range(NF):
                        nc.scalar.copy(
                            out=den_sb[:, f * FCH : (f + 1) * FCH],
                            in_=ops[f][C : C + 1, :],
                        )
                    rcp = work.tile([1, N], F32, tag="rcp", bufs=1)
                    nc.vector.reciprocal_approx_fast(out=rcp, in_=den_sb)
                    rcp16 = work.tile([1, N], BF16, tag="rcp16", bufs=1)
                    nc.vector.tensor_copy(out=rcp16, in_=rcp)
                    rcpb = work.tile([C, N], BF16, tag="rcpb", bufs=2)
                    nc.gpsimd.partition_broadcast(rcpb[:], rcp16[:])
                    if dbg_here:
                        nc.sync.dma_start(out=drc[:], in_=rcp)
                        drbx = work.tile([C, N], F32, tag="dbgt", bufs=1, name="drbx")
                        nc.vector.tensor_copy(out=drbx, in_=rcpb)
                        nc.sync.dma_start(out=drb[:], in_=drbx)
                    num_sb = work.tile([C, N], BF16, tag="num")
                    for f in range(NF):
                        nc.scalar.copy(
                            out=num_sb[:, f * FCH : (f + 1) * FCH], in_=ops[f][:C, :]
                        )
                    if dbg_here:
                        dnmx = work.tile([C, N], F32, tag="dbgt", bufs=1, name="dnmx")
                        nc.vector.tensor_copy(out=dnmx, in_=num_sb)
                        nc.sync.dma_start(out=dnm[:], in_=dnmx)
                    if k == 0:
                        nc.vector.tensor_mul(out=accT, in0=num_sb, in1=rcpb)
                    else:
                        tsc = work.tile([C, N], BF16, tag="tsc", bufs=1)
                        nc.vector.tensor_mul(out=tsc, in0=num_sb, in1=rcpb)
                        nc.vector.tensor_add(out=accT, in0=accT, in1=tsc)
                if debug and j == 0:
                    daccx = work.tile([C, N], F32, tag="dbgt", bufs=1)
                    nc.vector.tensor_copy(out=daccx, in_=accT)
                    nc.sync.dma_start(out=dacc[:], in_=daccx)
                # elu(accT) = relu(a) + exp(min(a,0)) - 1
                mn = work.tile([C, N], BF16, tag="mn", bufs=1)
                nc.vector.tensor_scalar_min(mn, accT, 0.0)
                ex = work.tile([C, N], BF16, tag="ex", bufs=1)
                nc.scalar.activation(out=ex, in_=mn, func=AF.Exp)
                rl = work.tile([C, N], BF16, tag="rl", bufs=1)
                nc.scalar.activation(out=rl, in_=accT, func=AF.Relu)
                er = work.tile([C, N], BF16, tag="er", bufs=1)
                nc.vector.tensor_add(out=er, in0=ex, in1=rl)
                elu = work.tile([C, N], BF16, tag="elu", bufs=2)
                nc.vector.tensor_scalar_add(elu, er, -1.0)
                nc.sync.dma_start(out=ag_in[j], in_=elu)
                nc.gpsimd.collective_compute(
                    "AllGather", ALU.bypass,
                    replica_groups=[list(range(n_cores))],
                    ins=[ag_in[j : j + 1]], outs=[agos[j][:]],
                )
            mk.release()

            # ---------------- tail: replicated ---------------------------
            tl = tc.alloc_tile_pool(name="tl", bufs=1)
            av_sb = tl.tile([C, NSLICE, N], BF16)
            h2_sb = tl.tile([C, B * T2, N], BF16)
            for b in range(B):
                for s in range(b * T1, (b + 1) * T1):
                    t0a, s0 = agslot(2 * s)
                    t1a, s1 = agslot(2 * s + 1)
                    a0 = work.tile([C, N], BF16, tag="ga0", bufs=1)
                    nc.sync.dma_start(out=a0, in_=t0a[s0])
                    a1 = work.tile([C, N], BF16, tag="ga1", bufs=1)
                    nc.sync.dma_start(out=a1, in_=t1a[s1])
                    nc.vector.tensor_add(out=av_sb[:, s, :], in0=a0, in1=a1)
                for t2 in range(T2):
                    glu_conv(
                        w2_sb, r05_sb,
                        lambda tau, b=b, t2=t2: av_sb[:, b * T1 + t2 + tau, :],
                        h2_sb[:, b * T2 + t2, :], pe_evac_dve=True,
                    )
            # T2: stats for all pairs -> batch scalar math (sqrt)
            pairs = B * T2
            stat_sb = work.tile([1, 2 * pairs], F32, tag="stats", bufs=1)
            for p in range(pairs):
                h2 = h2_sb[:, p, :]
                sums = work.tile([C, 1], F32, tag="sums")
                nc.vector.tensor_reduce(out=sums, in_=h2, axis=AX.X, op=ALU.add)
                sq = work.tile([C, N], BF16, tag="sqr", bufs=1)
                nc.vector.tensor_mul(out=sq, in0=h2, in1=h2)
                sqs = work.tile([C, 1], F32, tag="sqs")
                nc.vector.tensor_reduce(out=sqs, in_=sq, axis=AX.X, op=ALU.add)
                pair2 = work.tile([C, 2], F32, tag="pair2")
                nc.scalar.copy(out=pair2[:, 0:1], in_=sums)
                nc.scalar.copy(out=pair2[:, 1:2], in_=sqs)
                totp = ps_s.tile([1, 2], F32, tag="alp", bufs=1, name="totp")
                nc.tensor.matmul(out=totp, lhsT=ones64x1, rhs=pair2,
                                 start=True, stop=True)
                nc.scalar.copy(out=stat_sb[:, 2 * p : 2 * p + 2], in_=totp)
            # batch: mu, var, rstd, -mu*rstd  (strided [1,pairs] views)
            mu = work.tile([1, pairs], F32, tag="mu", bufs=1)
            nc.scalar.activation(out=mu, in_=stat_sb[0:1, 0 : 2 * pairs : 2],
                                 func=AF.Identity, scale=1.0 / NC_ELEMS)
            es = work.tile([1, pairs], F32, tag="es", bufs=1)
            nc.scalar.activation(out=es, in_=stat_sb[0:1, 1 : 2 * pairs : 2],
                                 func=AF.Identity, scale=1.0 / NC_ELEMS)
            musq = work.tile([1, pairs], F32, tag="musq", bufs=1)
            nc.vector.tensor_mul(out=musq, in0=mu, in1=mu)
            varp = work.tile([1, pairs], F32, tag="varp", bufs=1)
            nc.vector.tensor_sub(out=varp, in0=es, in1=musq)
            sd = work.tile([1, pairs], F32, tag="sd", bufs=1)
            nc.scalar.activation(out=sd, in_=varp, func=AF.Sqrt, bias=eps_sb)
            rstd = work.tile([1, pairs], F32, tag="rstd", bufs=1)
            nc.vector.reciprocal_approx_fast(out=rstd, in_=sd)
            nmr = work.tile([1, pairs], F32, tag="nmr", bufs=1)
            nc.vector.tensor_mul(out=nmr, in0=mu, in1=rstd)
            nc.scalar.mul(nmr, nmr, -1.0)
            sb2 = work.tile([1, 2 * pairs], F32, tag="sb2", bufs=1)
            nc.scalar.copy(out=sb2[:, 0 : 2 * pairs : 2], in_=rstd)
            nc.scalar.copy(out=sb2[:, 1 : 2 * pairs : 2], in_=nmr)
            bcp = ps_s.tile([C, 2 * pairs], F32, tag="alp", bufs=1, name="bcp")
            nc.tensor.matmul(out=bcp, lhsT=ones1x64f, rhs=sb2, start=True, stop=True)
            bc = work.tile([C, 2 * pairs], F32, tag="bc", bufs=1)
            nc.scalar.copy(out=bc, in_=bcp)
            # T3: normalize + (gamma, beta) + out
            for p in range(pairs):
                b, t2 = p // T2, p % T2
                og = work.tile([C, N], F32, tag="og", bufs=1)
                nc.scalar.activation(
                    out=og, in_=h2_sb[:, p, :], func=AF.Identity,
                    scale=bc[:, 2 * p : 2 * p + 1], bias=bc[:, 2 * p + 1 : 2 * p + 2],
                )
                if not triv_gb:
                    nc.vector.tensor_mul(out=og, in0=og, in1=gb_sb[:, 0, :])
                    nc.vector.tensor_add(out=og, in0=og, in1=gb_sb[:, 1, :])
                nc.sync.dma_start(out=out[b, t2], in_=og)
            tl.release()
    if not nc.is_finalized():
        nc.finalize()
    return nc


def _prep(inputs, n_cores):
    bf16 = ml_dtypes.bfloat16
    fp8 = ml_dtypes.float8_e4m3fn
    x = np.asarray(inputs["x"], np.float32)
    supports = np.asarray(inputs["supports"], np.float32)
    atten = np.asarray(inputs["atten_supports"], np.float32)
    w_t1 = np.asarray(inputs["w_t1"], np.float32)
    Wt = np.asarray(inputs["Wt"], np.float32)
    Wl = np.asarray(inputs["Wl"], np.float32)
    Wr = np.asarray(inputs["Wr"], np.float32)
    w_t2 = np.asarray(inputs["w_t2"], np.float32)
    gamma = np.asarray(inputs["gamma"], np.float32)
    beta = np.asarray(inputs["beta"], np.float32)

    xT = np.ascontiguousarray(x.transpose(0, 1, 3, 2)).astype(bf16)  # [B,T,C,N]
    w1T = np.ascontiguousarray(w_t1[:, :, :, 0].transpose(2, 1, 0)).astype(bf16)
    w2T = np.ascontiguousarray((0.5 * w_t2[:, :, :, 0]).transpose(2, 1, 0)).astype(bf16)
    eye = np.eye(C, dtype=np.float32)
    res1 = np.concatenate([eye, 0 * eye], axis=1).astype(bf16)          # [C,2C]
    res05 = np.concatenate([0.5 * eye, 0 * eye], axis=1).astype(bf16)
    mrel = (atten != 0).astype(np.float32)
    msup = (supports != 0).astype(np.float32)
    uni = np.minimum(mrel[0] + mrel[1] + msup, 1.0)                      # [K1,N,N]
    mrelT = np.ascontiguousarray(mrel.transpose(0, 2, 1)).astype(bf16)
    msupT = np.ascontiguousarray(msup.transpose(0, 2, 1)).astype(bf16)
    unegT = np.ascontiguousarray((uni - 1.0).transpose(0, 2, 1)).astype(fp8)
    uT = np.ascontiguousarray(uni.transpose(0, 2, 1)).astype(fp8)
    gbT = np.stack([gamma[0, 0].T, beta[0, 0].T]).astype(bf16)

    in_maps = []
    for c in range(n_cores):
        units = [4 * c, 4 * c + 1, 4 * c + 2, 4 * c + 3, 32 + c]
        slices = sorted({u // 2 for u in units})
        assert len(slices) == NSL
        xwa = np.empty((NSL, C, KT, N), bf16)
        for i, s in enumerate(slices):
            b, t1 = s // T1, s % T1
            for tau in range(KT):
                xwa[i, :, tau] = xT[b, t1 + tau]
        wlrA = np.empty((UPC, K1, C, K1), np.float32)
        wrtA = np.empty((UPC, K1, C, K1 + C), np.float32)
        for j, u in enumerate(units):
            h = u % 2
            assert u // 2 == slices[J2S[j]]
            for k in range(K1):
                wlrA[j, k] = Wt[h, k] @ Wl[h, k].T
                wrtA[j, k, :, :K1] = Wt[h, k] @ Wr[h, k].T
                wrtA[j, k, :, K1:] = Wt[h, k]
        in_maps.append(dict(
            xw=xwa, w1T=w1T, w2T=w2T, res1=res1, res05=res05,
            wlr=wlrA.astype(bf16), wrt=wrtA.astype(bf16),
            mrelT=mrelT, msupT=msupT, unegT=unegT, uT=uT, gbT=gbT,
        ))
    triv = bool(np.all(gamma == 1.0) and np.all(beta == 0.0))
    return in_maps, triv


LAST = None


def kernel(**inputs):
    global LAST
    from concourse.bass_utils import run_bass_kernel_spmd

    in_maps, triv = _prep(inputs, N_CORES)
    key = (N_CORES, triv)
    if key not in _cache:
        _cache[key] = _build(N_CORES, triv_gb=triv)
    nc = _cache[key]
    res = run_bass_kernel_spmd(nc, in_maps, list(range(N_CORES)))
    LAST = res
    o = np.asarray(res.results[0]["out"], np.float32)
    return np.ascontiguousarray(o.transpose(0, 1, 3, 2))
